# revision 11
# baseline (speedup 1.0000x reference)
"""Trainium2 kernel for nn_DistanceLoss (retrieval_knn, bs=1, N=16384).

reference semantics (sym branch, model_index in (0,)):
    p = R @ pts_model + t                      # (N, 3) predicted points
    d2[i, j] = ||p_i - g_j||^2                 # (N, N) vs ground-truth points
    loss = mean_i sqrt(min_j d2[i, j])         # scalar, shape (1,)

Key identity: sqrt(d2[i, argmin_j]) == sqrt(min_j d2[i, j]), so no
argmin/gather is needed — only a min-reduction over the distance matrix:
    min_j d2[i, j] = p_i^2 + min_j (g_j^2 - 2 p_i . g_j)

Device work (the O(N^2) part), sharded over 8 cores on the pred axis:
  - PE: S[i, j] = -2 p_i . g_j + g_j^2 as a K=11 matmul: each fp32 operand
    is split into fp16 hi/lo halves (lo scaled by 2^6 to dodge fp16
    subnormals, compensated on the other operand) so every partial product
    is exact in the fp32 PSUM accumulate; abs error ~1e-5.
  - Reduction of the 2048x16384 S-slice per core: PSUM is drained in
    [128, 1024] tiles (2 banks x 4 = all 8 banks; 4-deep rotation keeps
    the PE two tiles ahead so the DVE never stalls on fills). ScalarE
    copies every even tile to SBUF; a custom fused DVE op
    (MIN_TT_REDUCE_ANT: out = min(in0, in1), accum_out = min(s0,
    reduce_min(out))) consumes (odd PSUM tile, even SBUF tile) pairs at
    1 result/cycle = 2 source elements/cycle — the DVE is the critical
    engine at ~93% occupancy.
  - gt points are permuted on the host into four 4096-column slabs, one
    per PE row-group (partition offsets 0/32/64/96), so features load as
    4 contiguous DMAs (4x less input traffic than replicating full rhs
    per row-group). Slab DMAs are split head/tail so the first matmul
    only waits on ~55KB.
Host work (O(N)): pose transform, fp16 feature split, gt slab packing,
final p^2 add + sqrt + mean in float64, and the trivial non-symmetric
branch.
"""

import numpy as np

N_PTS = 16384
N_CORES = 8
SYM_LIST = (0,)

PRED_PER_CORE = N_PTS // N_CORES          # 2048
N_BLOCKS = PRED_PER_CORE // 128           # 16 pred blocks of 128 rows
GROUP = 1024                              # gt points per PSUM tile (2 banks)
N_GROUPS = N_PTS // GROUP                 # 16 tiles -> 8 (even, odd) pairs
N_PAIRS = N_GROUPS // 2                   # 8 fused DVE ops per block
K_ROWS = 11                               # fp16 split rows (3 per coord + 2)
LO_SCALE = np.float32(64.0)               # 2^6 subnormal-dodge scale
SLAB = 4096                               # gt cols per row-group slab
HEAD = 2048                               # slab cols loaded by the head DMA

TRACE = False          # test.py sets True to capture a profiled run
LAST_RESULT = None     # BassKernelResults of the most recent device run

_COMPILED = None


def _register_min_ttr():
    """Register a custom fused DVE op:
        out = min(in0, in1);  accum_out = min(reduce_min(out), s0)
    One DVE instruction consumes TWO tiles at 1 result/cycle — 2x the
    throughput of tensor_reduce for the same reduction work. (The native
    TENSOR_TENSOR_REDUCE opcode crashes this runtime's exec unit; the
    table-driven custom-DVE path works.)"""
    from concourse.dve_spec import Spec, Src0, Src1, C0, minn, lower, _has_src1
    from concourse.dve_uop import DveOpSpec
    from concourse import dve_ops

    name = "MIN_TT_REDUCE_ANT"
    for o in dve_ops.OPS:
        if o.name == name:
            return o

    def _ref(in0, in1, c0, c1, c2):
        b = np.minimum(in0.astype(np.float32), in1.astype(np.float32))
        acc = np.minimum(
            np.float32(c0), b.reshape(b.shape[0], -1).min(axis=-1, keepdims=True)
        )
        return b, acc

    spec = Spec(body=minn(Src0, Src1), accum=minn, accum_init=C0, reference=_ref)
    row = max(dve_ops._SUB_OPCODE_FOR_NAME.values()) + 1
    dve_ops._SUB_OPCODE_FOR_NAME[name] = row
    shas = {}
    for ver in ("v3", "v4"):
        uops = lower(spec, ver=ver)
        shas[ver] = DveOpSpec(
            name=name, opcode=row, uops=uops, rd1_en=_has_src1(spec)
        ).sha(ver)
    op = dve_ops.DveOp(name, spec, subdim=False, uops_sha=shas)
    dve_ops.OPS.append(op)
    dve_ops.CUSTOM_DVE_SPECS[name] = spec
    return op


def _build_module():
    import concourse.bacc as bacc
    import concourse.tile as tile
    import concourse.mybir as mybir

    f16 = mybir.dt.float16
    f32 = mybir.dt.float32
    min_ttr = _register_min_ttr()

    nc = bacc.Bacc(
        "TRN2", target_bir_lowering=False, debug=False, num_devices=N_CORES
    )
    lhsT = nc.dram_tensor("lhsT", [K_ROWS, PRED_PER_CORE], f16, kind="ExternalInput")
    # slab r holds gt chunks r, r+4, r+8, ... (512-col chunks): tile q of a
    # block reads slab_{(2q)%4}[:, (q//2)*512 : ...] and slab_{(2q+1)%4}.
    slabs_in = [
        nc.dram_tensor(f"slab{r}", [K_ROWS, SLAB], f16, kind="ExternalInput")
        for r in range(4)
    ]
    # [128, 8] of per-pair partial mins per block; host min-reduces pairs
    out = nc.dram_tensor(
        "out", [128, N_BLOCKS * N_PAIRS], f32, kind="ExternalOutput"
    )

    with tile.TileContext(nc) as tc:
        with (
            tc.tile_pool(name="consts", bufs=1) as consts,
            tc.tile_pool(name="scrp", bufs=6) as scrp,
            tc.tile_pool(name="ttrop", bufs=4) as ttrop,
            tc.tile_pool(name="accp", bufs=4) as accp,
            tc.tile_pool(name="ps", bufs=4, space="PSUM") as psp,
        ):
            # pred features replicated at partition offsets 0/32/64/96 so
            # four K=11 matmuls run CONCURRENTLY in distinct PE row-groups;
            # gt slabs land at matching offsets (slab r at offset 32r).
            # Separate tiles per row-group so the first matmul's semaphore
            # only covers the two DMAs it actually reads.
            lhs_t = [
                consts.tile([32 * r + K_ROWS, PRED_PER_CORE], f16, name=f"lhs{r}")
                for r in range(4)
            ]
            slab_head = [
                consts.tile([32 * r + K_ROWS, HEAD], f16, name=f"sh{r}")
                for r in range(4)
            ]
            slab_tail = [
                consts.tile([32 * r + K_ROWS, SLAB - HEAD], f16, name=f"st{r}")
                for r in range(4)
            ]
            # warm-up FIRST: the ACT table load (~2.7us) must be off the
            # critical path, so ScalarE's opening instruction is a
            # self-contained scale-by-zero (reads nothing).
            warm = scrp.tile([128, 32], f32, tag="warm")
            warm2 = scrp.tile([128, 32], f32, tag="warm")
            wacc = accp.tile([128, 1], f32, tag="wacc")
            nc.scalar.mul(warm2[:], warm2[:], 0.0)
            nc.vector.memset(warm[:], 0.0)
            nc.vector._custom_dve(
                min_ttr, out=warm[:], in0=warm[:], in1=warm[:],
                s0=3.0e38, accum_out=wacc[:],
            )

            # every engine issues DMAs on its OWN hardware queue; all input
            # DMAs on one engine serialize. Spread across the idle SP +
            # GPSIMD sequencers (ScalarE must stay free: its first PSUM
            # copy is on the critical path). The first fused pair reads
            # row-groups 0/1 only, so those four DMAs go first on the
            # HWDGE (sync) queue; row-groups 2/3 ride gpsimd.
            for r in range(4):
                p0 = 32 * r
                e = nc.sync if r < 2 else nc.gpsimd
                e.dma_start(lhs_t[r][p0 : p0 + K_ROWS, :], lhsT[:])
                e.dma_start(
                    slab_head[r][p0 : p0 + K_ROWS, :], slabs_in[r][:, :HEAD]
                )
            for r in range(4):
                p0 = 32 * r
                e = nc.sync if r < 2 else nc.gpsimd
                e.dma_start(
                    slab_tail[r][p0 : p0 + K_ROWS, :], slabs_in[r][:, HEAD:]
                )

            def mm_tile(ps, b, q):
                """One PSUM tile [128, 1024]: tile q lives wholly in slab
                q%4 at cols (q//4)*1024, so a fused pair (q, q+4) only
                depends on one slab's DMAs. Both matmuls share a PE
                row-group; consecutive pairs rotate row-groups so four
                matmuls stay in flight across the PSUM rotation."""
                r = q % 4
                p0 = 32 * r
                base = (q // 4) * 1024
                for t in range(2):
                    col = base + t * 512
                    if col < HEAD:
                        src = slab_head[r][p0 : p0 + K_ROWS, col : col + 512]
                    else:
                        src = slab_tail[r][
                            p0 : p0 + K_ROWS, col - HEAD : col - HEAD + 512
                        ]
                    nc.tensor.matmul(
                        ps[:, t * 512 : (t + 1) * 512],
                        lhs_t[r][p0 : p0 + K_ROWS, b * 128 : (b + 1) * 128],
                        src,
                        start=True,
                        stop=True,
                        tile_position=(p0, 0),
                    )

            # pair (q, q+4) lives in a single slab: fuse #1 only waits on
            # the lhs0 + head0 DMAs. Consecutive pairs rotate slabs.
            pair_list = [
                (base + r, base + r + 4)
                for base in range(0, N_GROUPS, 8)
                for r in range(4)
            ]
            half = N_BLOCKS * N_PAIRS // 2
            # two acc tiles so the mid-kernel output DMA never blocks
            # later accumulator writes (no shared-tile WAR)
            acc_t = [
                accp.tile([128, half], f32, tag="accs", name=f"accs{h}")
                for h in range(2)
            ]
            for b in range(N_BLOCKS):
                accs = acc_t[b // (N_BLOCKS // 2)]
                bb = b % (N_BLOCKS // 2)
                for k, (qa, qb) in enumerate(pair_list):
                    # a-tile: ScalarE copies PSUM -> SBUF
                    ps_a = psp.tile([128, GROUP], f32, tag="ps")
                    mm_tile(ps_a, b, qa)
                    scr = scrp.tile([128, GROUP], f32, tag="scr")
                    nc.scalar.copy(scr[:], ps_a[:])
                    # b-tile: consumed straight from PSUM by the fused op
                    ps_b = psp.tile([128, GROUP], f32, tag="ps")
                    mm_tile(ps_b, b, qb)
                    ttr_out = ttrop.tile([128, GROUP], f32, tag="ttro")
                    nc.vector._custom_dve(
                        min_ttr,
                        out=ttr_out[:],
                        in0=ps_b[:],
                        in1=scr[:],
                        s0=3.0e38,
                        accum_out=accs[:, bb * N_PAIRS + k : bb * N_PAIRS + k + 1],
                    )
                if b == N_BLOCKS // 2 - 1:
                    # first half of the accumulators ships mid-kernel so the
                    # final DMA only carries 256B per partition
                    nc.sync.dma_start(out[:, :half], acc_t[0][:])
            nc.sync.dma_start(out[:, half:], acc_t[1][:])
    nc.compile()
    return nc


def _get_module():
    global _COMPILED
    if _COMPILED is None:
        _COMPILED = _build_module()
    return _COMPILED


def _split_f16(x):
    """x (fp32) -> (hi, lo*2^6) fp16 pair with exact-product semantics."""
    hi = x.astype(np.float16)
    lo = ((x - hi.astype(np.float32)) * LO_SCALE).astype(np.float16)
    return hi, lo


def kernel(pred_R, pred_t, pts_model, pts_gt, model_index):
    global LAST_RESULT
    pred_R = np.asarray(pred_R, dtype=np.float32)
    pred_t = np.asarray(pred_t, dtype=np.float32)
    pts_model = np.asarray(pts_model, dtype=np.float32)
    pts_gt = np.asarray(pts_gt, dtype=np.float32)

    # pose transform (O(N), host): p[b,n,:] = R[b] @ model[b,n,:] + t[b]
    p = np.einsum("bij,bnj->bni", pred_R, pts_model) + pred_t[:, None, :]

    if int(model_index) not in SYM_LIST:
        diff = (p - pts_gt).astype(np.float64)
        loss = np.mean(np.sqrt(np.sum(diff * diff, axis=2)), axis=1)
        return loss.astype(np.float32)

    p = p[0]                       # (N, 3) queries
    g = pts_gt[0].astype(np.float32)   # (N, 3) references

    # features: S[i,j] = sum_k lhsT[k,i] * rhs[k,j] = -2 p.g + g^2
    a = -2.0 * p                                   # (N, 3)
    ah, al = _split_f16(a)
    gh, gl = _split_f16(g)
    c = (g.astype(np.float64) ** 2).sum(axis=1).astype(np.float32)   # g^2
    ch, cl = _split_f16(c)
    inv = np.float32(1.0) / LO_SCALE

    ones = np.ones(N_PTS, np.float16)
    # per coord: (Ah,Gh), (Al*64, Gh/64), (Ah/64, Gl*64); then (1,Ch), (1/64, Cl*64)
    lhs_rows, rhs_rows = [], []
    for ci in range(3):
        ahc = ah[:, ci]
        ghc = gh[:, ci]
        lhs_rows += [ahc, al[:, ci], (ahc.astype(np.float32) * inv).astype(np.float16)]
        rhs_rows += [ghc, (ghc.astype(np.float32) * inv).astype(np.float16), gl[:, ci]]
    lhs_rows += [ones, (ones.astype(np.float32) * inv).astype(np.float16)]
    rhs_rows += [ch, cl]
    lhs_full = np.stack(lhs_rows)                  # (11, N) fp16
    rhs_full = np.stack(rhs_rows)                  # (11, N) fp16

    # slab r = tiles r, r+4, r+8, r+12 (each tile = two 512-chunks), so a
    # fused pair (q, q+4) reads one contiguous 2048-col slab half
    rhs_chunked = rhs_full.reshape(K_ROWS, 32, 512)
    slabs = [
        np.ascontiguousarray(
            rhs_chunked[:, [2 * r + o + 8 * s for s in range(4) for o in (0, 1)], :]
            .reshape(K_ROWS, SLAB)
        )
        for r in range(4)
    ]

    nc = _get_module()
    from concourse.bass_utils import run_bass_kernel_spmd

    in_maps = []
    for core in range(N_CORES):
        sl = slice(core * PRED_PER_CORE, (core + 1) * PRED_PER_CORE)
        im = {"lhsT": np.ascontiguousarray(lhs_full[:, sl])}
        for r in range(4):
            im[f"slab{r}"] = slabs[r]
        in_maps.append(im)
    kw = {}
    if TRACE:
        kw = {"trace": True, "trace_cores": list(range(N_CORES))}
    res = run_bass_kernel_spmd(nc, in_maps, core_ids=list(range(N_CORES)), **kw)
    LAST_RESULT = res

    # assemble: out[p, b*8+k] = pair-k partial min for pred index
    # core*2048 + b*128 + p; min over pairs on host
    min_s = np.concatenate(
        [
            res.results[core]["out"]
            .reshape(128, N_BLOCKS, N_PAIRS)
            .min(axis=2)
            .T.reshape(-1)
            for core in range(N_CORES)
        ]
    ).astype(np.float64)
    p2 = (p.astype(np.float64) ** 2).sum(axis=1)
    d2 = np.maximum(p2 + min_s, 0.0)
    loss = np.mean(np.sqrt(d2))
    return np.array([loss], dtype=np.float32)
